# revision 44
# baseline (speedup 1.0000x reference)
"""Multi-head causal attention (B=2, S=2048, D=1024, H=16, DK=DV=64) on 8 Trainium2
NeuronCores.

Sharding: 2-way batch x 4-way head-group. Core i handles batch i//4 and heads
[4*(i%4), 4*(i%4)+4). Each core projects q/k/v for its head group, runs causal
attention, and computes a partial output projection through its row-block of Wo.
The 4 partial outputs per batch are summed on the host (the all-reduce of the
row-sharded Wo output).

v2 design notes (all driven by the HW p-state behavior: the PE clock ramps
0.65->1.2->2.4GHz only under continuous execution and any ~1us gap resets it):
- Everything is bf16 (inputs converted on host): halves DMA, full PE rate,
  and bf16 avoids the fp32r small-moving-dim penalty.
- One continuous PE instruction stream: attention chains are interleaved with
  projection groups / output-projection groups / rank-1 broadcasts as fillers
  so the PE never idles and exp latency is hidden.
- ACT (scalar) engine runs ONLY Exp (single table load, preloaded early).
- Masked work is skipped: scores/exp/ov all restrict to cols >= lo on
  diagonal tiles; no zero-fill needed.
- Softmax denominators come free from an all-ones column in vaug; recip via
  single-instruction reciprocal_approx_fast (DVE), broadcast via a rank-1 PE
  matmul, applied with one DVE multiply per (chunk, head).
- PSUM budget (8 banks): 4 ov + 3 sc (oproj shares sc) + 1 aux (proj groups,
  transposes, rank-1, warmup dummies).
- During the initial DMA-bound window the PE runs dummy transposes to ramp
  and hold the clock so the first attention chain starts at full speed.
"""
import sys

sys.path.insert(0, "/opt/trn_rl_repo")
import numpy as np

B, S, D = 2, 2048, 1024
H, DK, DV = 16, 64, 64
NCORES = 8
HG = 4          # head-group cores per batch
HPC = H // HG   # heads per core
HDC = HPC * DK  # 256 projection cols per core
P = 128         # partitions
CH = 512        # q-chunk size
VW = DV + 1     # v_aug width per head
NCH = S // CH   # 4 chunks
NST = S // P    # 16 s-tiles
ND = D // P     # 8 d-tiles
NM = HDC // P   # 2 head-pair tiles

PRE_DUMMIES = 24     # PE warmup transposes before any real work
W0_DUMMIES = (6, 4, 4)  # dummies between dd-matmuls for first V/K/Q group
DEBUG = False        # add intermediate-dump outputs


def build(nc, tile, mybir):
    BF16 = mybir.dt.bfloat16
    F32 = mybir.dt.float32
    F32R = mybir.dt.float32r
    Exp = mybir.ActivationFunctionType.Exp

    xvT = nc.dram_tensor("xvT", [D, S], BF16, kind="ExternalInput").ap()
    xkT = nc.dram_tensor("xkT", [D, S], BF16, kind="ExternalInput").ap()
    xqT = nc.dram_tensor("xqT", [D, S], BF16, kind="ExternalInput").ap()
    wqkv = nc.dram_tensor("wqkv", [D, 3 * HDC], BF16, kind="ExternalInput").ap()
    wo = nc.dram_tensor("wo", [HDC, D], BF16, kind="ExternalInput").ap()
    maskA = nc.dram_tensor("maskA", [P, P], BF16, kind="ExternalInput").ap()
    onesb = nc.dram_tensor("onesb", [1, P], BF16, kind="ExternalInput").ap()
    onesp = nc.dram_tensor("onesp", [P, HPC], BF16, kind="ExternalInput").ap()
    ident = nc.dram_tensor("ident", [P, P], F32R, kind="ExternalInput").ap()
    out = nc.dram_tensor("out", [S, D], BF16, kind="ExternalOutput").ap()
    dbg = None
    if DEBUG:
        dbg = {n: nc.dram_tensor(f"dbg_{n}", [P, S], BF16,
                                 kind="ExternalOutput").ap()
               for n in ("qT0", "kT0", "oU0", "oN0")}
        dbg["den"] = nc.dram_tensor("dbg_den", [16, CH], F32,
                                    kind="ExternalOutput").ap()
        dbg["rec"] = nc.dram_tensor("dbg_rec", [16, CH], F32,
                                    kind="ExternalOutput").ap()

    with tile.TileContext(nc) as tc:
        from contextlib import ExitStack
        with ExitStack() as ctx:
            wp = ctx.enter_context(tc.tile_pool(name="wp", bufs=1))
            exp_ = ctx.enter_context(tc.tile_pool(name="exp", bufs=12))
            sp = ctx.enter_context(tc.tile_pool(name="sp", bufs=2))
            obp = ctx.enter_context(tc.tile_pool(name="obp", bufs=3))
            sc_ps = ctx.enter_context(tc.tile_pool(name="sc_ps", bufs=3, space="PSUM"))
            ov_ps = ctx.enter_context(tc.tile_pool(name="ov_ps", bufs=4, space="PSUM"))
            aux_ps = ctx.enter_context(tc.tile_pool(name="aux_ps", bufs=1, space="PSUM"))

            # ---- persistent SBUF tiles ----
            idt = wp.tile([P, P], F32R, name="idt")
            mA = wp.tile([P, P], BF16, name="mA")
            onb = wp.tile([1, P], BF16, name="onb")
            onp = wp.tile([P, HPC], BF16, name="onp")
            wqkv_t = [wp.tile([P, 3 * HDC], BF16, name=f"wqkv{i}") for i in range(ND)]
            wo_t = [wp.tile([P, D], BF16, name=f"wo{i}") for i in range(NM)]
            xts = {tn: [[wp.tile([P, 2 * CH], BF16, name=f"x{tn}_{hf}_{dd}")
                         for dd in range(ND)] for hf in range(2)]
                   for tn in ("v", "k", "q")}
            qT = [wp.tile([P, S], BF16, name=f"qT{m}") for m in range(NM)]
            kT = [wp.tile([P, S], BF16, name=f"kT{m}") for m in range(NM)]
            oU = [wp.tile([P, S], BF16, name=f"oU{m}") for m in range(NM)]
            oN = [wp.tile([P, S], BF16, name=f"oN{m}") for m in range(NM)]
            vaug = [wp.tile([P, HPC * VW], BF16, name=f"vaug{t}") for t in range(NST)]
            vTs = [wp.tile([P, CH], F32R, name=f"vTs{m}") for m in range(NM)]
            scr = wp.tile([1, 16], BF16, name="scr")

            # ---- DMA issue order (SP queue = strict FIFO priority) ----
            nc.sync.dma_start(idt[:], ident[:, :])
            nc.sync.dma_start(mA[:], maskA[:, :])
            nc.sync.dma_start(onb[:], onesb[:, :])
            nc.sync.dma_start(onp[:], onesp[:, :])
            xsrc = {"v": xvT, "k": xkT, "q": xqT}

            def dma_x(tn, hf, dd, eng):
                eng.dma_start(xts[tn][hf][dd][:],
                              xsrc[tn][dd * P:(dd + 1) * P,
                                       hf * 2 * CH:(hf + 1) * 2 * CH])

            for dd in range(ND):
                nc.sync.dma_start(wqkv_t[dd][:], wqkv[dd * P:(dd + 1) * P, :])
                dma_x("v", 0, dd, nc.sync)
            for dd in range(ND):
                dma_x("k", 0, dd, nc.gpsimd)   # parallel queue
                dma_x("q", 0, dd, nc.scalar)   # parallel queue
            for i in range(NM):
                nc.sync.dma_start(wo_t[i][:], wo[i * P:(i + 1) * P, :])
            for tn in ("v", "k", "q"):
                for dd in range(ND):
                    dma_x(tn, 1, dd, nc.sync)

            # ACT: preload the Exp table during the DMA window
            nc.scalar.activation(scr[:], mA[0:1, 0:16], Exp)
            # vaug all-ones denominator columns (Pool memset; verified good)
            for t in range(NST):
                nc.gpsimd.memset(vaug[t][:, DV::VW], 1.0)

            # ---- emission helpers ----
            def dummy():
                dum = sc_ps.tile([P, P], F32R, name="dum", tag="sc")
                nc.tensor.transpose(dum[:], idt[:], idt[:])

            WSEL = {"q": 0, "k": HDC, "v": 2 * HDC}

            def proj_group(tn, m, c, dums=0):
                """project x_tn chunk c through W block m -> dest cols."""
                pp = aux_ps.tile([P, CH], F32, name="pg", tag="aux")
                hf, sub = c // 2, (c % 2) * CH
                for dd in range(ND):
                    w = wqkv_t[dd][:, WSEL[tn] + m * P:WSEL[tn] + (m + 1) * P]
                    nc.tensor.matmul(pp[:], w,
                                     xts[tn][hf][dd][:, sub:sub + CH],
                                     start=(dd == 0), stop=(dd == ND - 1))
                    for _ in range(dums):
                        dummy()
                if tn == "v":
                    nc.vector.tensor_copy(vTs[m][:], pp[:])
                else:
                    dst = (qT if tn == "q" else kT)[m][:, c * CH:(c + 1) * CH]
                    nc.vector.tensor_copy(dst, pp[:])

            def vtrans(m, c, j):
                """one [128,128] block of vTs[m] -> natural layout in vaug."""
                st = 4 * c + j
                tp = aux_ps.tile([P, P], F32R, name="tp", tag="aux")
                nc.tensor.transpose(tp[:], vTs[m][:, j * P:(j + 1) * P], idt[:])
                dst = vaug[st][:, m * 2 * VW:(m * 2 + 2) * VW].rearrange(
                    "p (h x) -> p h x", x=VW)[:, :, 0:DV]
                src = tp[:].rearrange("p (h x) -> p h x", x=DV)
                nc.vector.tensor_copy(dst, src)

            def sc_unit(h, c, t):
                """scores tile t for head h, chunk c -> exp'd ex tile."""
                mi, ri = h // 2, (h % 2) * DK
                r = t - 4 * c
                lo = max(r, 0) * P
                scp = sc_ps.tile([P, CH], F32, name="scp", tag="sc")
                nc.tensor.matmul(
                    scp[:, lo:CH],
                    kT[mi][ri:ri + DK, t * P:(t + 1) * P],
                    qT[mi][ri:ri + DK, c * CH + lo:(c + 1) * CH],
                    start=True, stop=True)
                ex = exp_.tile([P, CH], BF16, name="ex", tag="ex")
                nc.scalar.activation(ex[:, lo:CH], scp[:, lo:CH], Exp)
                if r >= 0:
                    eng = nc.vector if h < 2 else nc.gpsimd
                    eng.tensor_mul(ex[:, lo:lo + P], ex[:, lo:lo + P], mA[:])
                return ex, lo

            def ov_unit(ovt, h, c, t, ex, lo, nt):
                nc.tensor.matmul(ovt[:, lo:CH], vaug[t][:, h * VW:(h + 1) * VW],
                                 ex[:, lo:CH], start=(t == 0), stop=(t == nt - 1))

            recbs = {}

            def tail_recip(ovt, c, h, on_act):
                """recip of the denominator row -> recb (bf16)."""
                dn = sp.tile([1, CH], F32, name="dnf", tag="dnf", bufs=4)
                if on_act:
                    nc.scalar.copy(dn[:], ovt[DV:DV + 1, :])
                else:
                    nc.vector.tensor_copy(dn[:], ovt[DV:DV + 1, :])
                recf = sp.tile([1, CH], F32, name="recf", tag="recf", bufs=4)
                nc.vector.reciprocal_approx_fast(out=recf[:], in_=dn[:])
                rb = sp.tile([1, CH], BF16, name="recb", tag="recb", bufs=8)
                if on_act:
                    nc.scalar.copy(rb[:], recf[:])
                else:
                    nc.vector.tensor_copy(rb[:], recf[:])
                recbs[(c, h)] = rb
                if DEBUG:
                    r = 4 * c + h
                    nc.sync.dma_start(dbg["den"][r:r + 1, :], dn[:])
                    nc.sync.dma_start(dbg["rec"][r:r + 1, :], recf[:])

            def tail_oU(ovt, c, h, on_act):
                """stash the unnormalized numerator."""
                mi, ri = h // 2, (h % 2) * DK
                dst = oU[mi][ri:ri + DK, c * CH:(c + 1) * CH]
                if on_act:
                    nc.scalar.copy(dst, ovt[0:DV, :])
                else:
                    nc.vector.tensor_copy(dst, ovt[0:DV, :])

            def rank1mult(c, h):
                mi, ri = h // 2, (h % 2) * DK
                rb = aux_ps.tile([DK, CH], F32, name="rb", tag="aux")
                nc.tensor.matmul(rb[:], onb[0:1, 0:DK], recbs[(c, h)][:],
                                 start=True, stop=True)
                sl = slice(c * CH, (c + 1) * CH)
                nc.vector.tensor_mul(oN[mi][ri:ri + DK, sl],
                                     oU[mi][ri:ri + DK, sl], rb[:])

            def oproj(c, j):
                st = 4 * c + j
                ob = obp.tile([P, D], BF16, name="ob", tag="ob")
                for n2 in range(D // CH):
                    pp = sc_ps.tile([P, CH], F32, name="pout", tag="sc")
                    for m in range(NM):
                        nc.tensor.matmul(pp[:], oN[m][:, st * P:(st + 1) * P],
                                         wo_t[m][:, n2 * CH:(n2 + 1) * CH],
                                         start=(m == 0), stop=(m == NM - 1))
                    nc.vector.tensor_copy(ob[:, n2 * CH:(n2 + 1) * CH], pp[:])
                nc.gpsimd.dma_start(out[st * P:(st + 1) * P, :], ob[:])

            # ---- W0: warmup + project chunk 0 ----
            for _ in range(PRE_DUMMIES):
                dummy()
            for i, tn in enumerate(("v", "k", "q")):
                proj_group(tn, 0, 0, dums=W0_DUMMIES[i])
                proj_group(tn, 1, 0, dums=2)
                if tn == "v":
                    for m in range(NM):
                        for j in range(4):
                            vtrans(m, 0, j)

            # ---- waves: chain(c) with interleaved fillers ----
            def wave_units(c):
                projs, rank1s, oprojs = [], [], []
                if c + 1 < NCH:
                    for tn in ("v", "k", "q"):
                        for m in range(NM):
                            projs.append(
                                lambda tn=tn, m=m: proj_group(tn, m, c + 1))
                        if tn == "v":
                            for m in range(NM):
                                for j in range(4):
                                    projs.append(
                                        lambda m=m, j=j: vtrans(m, c + 1, j))
                if c >= 1:
                    for h in range(HPC):
                        rank1s.append(lambda h=h: rank1mult(c - 1, h))
                    for j in range(4):
                        oprojs.append(lambda j=j: oproj(c - 1, j))
                return projs[:2] + rank1s + projs[2:] + oprojs

            for c in range(NCH):
                nt = 4 * c + 4
                last = c == NCH - 1
                units = wave_units(c)
                # hold back units to cover the chain tail's recip latency
                hold = 3 if last else 0
                done = 0
                ovts = [ov_ps.tile([DV + 1, CH], F32, name=f"ov{h}", tag="ov")
                        for h in range(HPC)]
                exq = []  # pending (t, [per-head (ex, lo)]) with lag 2
                for t in range(nt):
                    cur = [sc_unit(0, c, t), sc_unit(1, c, t)]
                    if len(exq) >= 2:
                        ot, oexs = exq.pop(0)
                        for h in range(HPC):
                            ov_unit(ovts[h], h, c, ot, *oexs[h], nt)
                    cur += [sc_unit(2, c, t), sc_unit(3, c, t)]
                    exq.append((t, cur))
                    want = min(len(units) * (t + 1) // nt, len(units) - hold)
                    while done < want:
                        units[done]()
                        done += 1
                for ot, oexs in exq:
                    for h in range(HPC):
                        ov_unit(ovts[h], h, c, ot, *oexs[h], nt)
                for h in range(HPC):
                    tail_recip(ovts[h], c, h, on_act=last)
                for h in range(HPC):
                    tail_oU(ovts[h], c, h, on_act=last)
                if last:
                    if done < len(units):
                        units[done]()
                        done += 1
                    for h in range(HPC):
                        rank1mult(c, h)
                while done < len(units):
                    units[done]()
                    done += 1

            # ---- drain ----
            for j in range(4):
                oproj(NCH - 1, j)
            if DEBUG:
                for n, t in (("qT0", qT[0]), ("kT0", kT[0]),
                             ("oU0", oU[0]), ("oN0", oN[0])):
                    nc.sync.dma_start(dbg[n][:, :], t[:])
    nc.compile()
    return nc


_NC_CACHE = {}
LAST_RESULT = None


def _get_nc():
    if "nc" not in _NC_CACHE:
        import concourse.tile as tile
        import concourse.mybir as mybir
        from concourse import bacc
        nc = bacc.Bacc("TRN2", target_bir_lowering=False, num_devices=NCORES)
        _NC_CACHE["nc"] = build(nc, tile, mybir)
    return _NC_CACHE["nc"]


def kernel(Q, K, V, Wq, Wk, Wv, Wo):
    import ml_dtypes
    from concourse.bass_utils import run_bass_kernel_spmd
    BF = ml_dtypes.bfloat16

    Q = np.asarray(Q, dtype=np.float32)
    K = np.asarray(K, dtype=np.float32)
    V = np.asarray(V, dtype=np.float32)
    Wq = np.asarray(Wq, dtype=np.float32) * np.float32(1.0 / np.sqrt(DK))
    Wk = np.asarray(Wk, dtype=np.float32)
    Wv = np.asarray(Wv, dtype=np.float32)
    Wo = np.asarray(Wo, dtype=np.float32)

    QT = [np.ascontiguousarray(Q[b].T).astype(BF) for b in range(B)]
    KT = [np.ascontiguousarray(K[b].T).astype(BF) for b in range(B)]
    VT = [np.ascontiguousarray(V[b].T).astype(BF) for b in range(B)]

    i = np.arange(P)[:, None]
    j = np.arange(P)[None, :]
    maskA = (j >= i).astype(BF)
    onesb = np.ones((1, P), dtype=BF)
    onesp = np.ones((P, HPC), dtype=BF)
    ident = np.eye(P, dtype=np.float32)

    in_maps = []
    for core in range(NCORES):
        b, g = core // HG, core % HG
        cs = slice(g * HDC, (g + 1) * HDC)
        in_maps.append({
            "xqT": QT[b], "xkT": KT[b], "xvT": VT[b],
            "wqkv": np.ascontiguousarray(
                np.concatenate([Wq[:, cs], Wk[:, cs], Wv[:, cs]],
                               axis=1)).astype(BF),
            "wo": np.ascontiguousarray(Wo[cs, :]).astype(BF),
            "maskA": maskA, "onesb": onesb, "onesp": onesp, "ident": ident,
        })

    nc = _get_nc()
    res = run_bass_kernel_spmd(nc, in_maps, core_ids=list(range(NCORES)))
    global LAST_RESULT
    LAST_RESULT = res

    acc = np.zeros((B, S, D), dtype=np.float64)
    for core in range(NCORES):
        acc[core // HG] += res.results[core]["out"].astype(np.float64)
    return acc.astype(np.float32)


# revision 46
# speedup vs baseline: 1.0531x; 1.0531x over previous
"""Multi-head causal attention (B=2, S=2048, D=1024, H=16, DK=DV=64) on 8 Trainium2
NeuronCores.

Sharding: 2-way batch x 4-way head-group. Core i handles batch i//4 and heads
[4*(i%4), 4*(i%4)+4). Each core projects q/k/v for its head group, runs causal
attention, and computes a partial output projection through its row-block of Wo.
The 4 partial outputs per batch are summed on the host (the all-reduce of the
row-sharded Wo output).

v2 design notes (all driven by the HW p-state behavior: the PE clock ramps
0.65->1.2->2.4GHz only under continuous execution and any ~1us gap resets it):
- Everything is bf16 (inputs converted on host): halves DMA, full PE rate,
  and bf16 avoids the fp32r small-moving-dim penalty.
- One continuous PE instruction stream: attention chains are interleaved with
  projection groups / output-projection groups / rank-1 broadcasts as fillers
  so the PE never idles and exp latency is hidden.
- ACT (scalar) engine runs ONLY Exp (single table load, preloaded early).
- Masked work is skipped: scores/exp/ov all restrict to cols >= lo on
  diagonal tiles; no zero-fill needed.
- Softmax denominators come free from an all-ones column in vaug; recip via
  single-instruction reciprocal_approx_fast (DVE), broadcast via a rank-1 PE
  matmul, applied with one DVE multiply per (chunk, head).
- PSUM budget (8 banks): 4 ov + 3 sc (oproj shares sc) + 1 aux (proj groups,
  transposes, rank-1, warmup dummies).
- During the initial DMA-bound window the PE runs dummy transposes to ramp
  and hold the clock so the first attention chain starts at full speed.
"""
import sys

sys.path.insert(0, "/opt/trn_rl_repo")
import numpy as np

B, S, D = 2, 2048, 1024
H, DK, DV = 16, 64, 64
NCORES = 8
HG = 4          # head-group cores per batch
HPC = H // HG   # heads per core
HDC = HPC * DK  # 256 projection cols per core
P = 128         # partitions
CH = 512        # q-chunk size
VW = DV + 1     # v_aug width per head
NCH = S // CH   # 4 chunks
NST = S // P    # 16 s-tiles
ND = D // P     # 8 d-tiles
NM = HDC // P   # 2 head-pair tiles

PRE_DUMMIES = 36     # PE warmup transposes before any real work
W0_DUMMIES = (6, 4, 4)  # dummies between dd-matmuls for first V/K/Q group
DEBUG = False        # add intermediate-dump outputs


def build(nc, tile, mybir):
    BF16 = mybir.dt.bfloat16
    F32 = mybir.dt.float32
    F32R = mybir.dt.float32r
    Exp = mybir.ActivationFunctionType.Exp

    xvT = nc.dram_tensor("xvT", [D, S], BF16, kind="ExternalInput").ap()
    xkT = nc.dram_tensor("xkT", [D, S], BF16, kind="ExternalInput").ap()
    xqT = nc.dram_tensor("xqT", [D, S], BF16, kind="ExternalInput").ap()
    wqkv = nc.dram_tensor("wqkv", [D, 3 * HDC], BF16, kind="ExternalInput").ap()
    wo = nc.dram_tensor("wo", [HDC, D], BF16, kind="ExternalInput").ap()
    maskA = nc.dram_tensor("maskA", [P, P], BF16, kind="ExternalInput").ap()
    onesb = nc.dram_tensor("onesb", [1, P], BF16, kind="ExternalInput").ap()
    onesp = nc.dram_tensor("onesp", [P, HPC], BF16, kind="ExternalInput").ap()
    ident = nc.dram_tensor("ident", [P, P], F32R, kind="ExternalInput").ap()
    out = nc.dram_tensor("out", [S, D], BF16, kind="ExternalOutput").ap()
    dbg = None
    if DEBUG:
        dbg = {n: nc.dram_tensor(f"dbg_{n}", [P, S], BF16,
                                 kind="ExternalOutput").ap()
               for n in ("qT0", "kT0", "oU0", "oN0")}
        dbg["den"] = nc.dram_tensor("dbg_den", [16, CH], F32,
                                    kind="ExternalOutput").ap()
        dbg["rec"] = nc.dram_tensor("dbg_rec", [16, CH], F32,
                                    kind="ExternalOutput").ap()

    with tile.TileContext(nc) as tc:
        from contextlib import ExitStack
        with ExitStack() as ctx:
            wp = ctx.enter_context(tc.tile_pool(name="wp", bufs=1))
            exp_ = ctx.enter_context(tc.tile_pool(name="exp", bufs=12))
            sp = ctx.enter_context(tc.tile_pool(name="sp", bufs=2))
            obp = ctx.enter_context(tc.tile_pool(name="obp", bufs=3))
            sc_ps = ctx.enter_context(tc.tile_pool(name="sc_ps", bufs=3, space="PSUM"))
            ov_ps = ctx.enter_context(tc.tile_pool(name="ov_ps", bufs=4, space="PSUM"))
            aux_ps = ctx.enter_context(tc.tile_pool(name="aux_ps", bufs=1, space="PSUM"))

            # ---- persistent SBUF tiles ----
            idt = wp.tile([P, P], F32R, name="idt")
            mA = wp.tile([P, P], BF16, name="mA")
            onb = wp.tile([1, P], BF16, name="onb")
            onp = wp.tile([P, HPC], BF16, name="onp")
            wqkv_t = [wp.tile([P, 3 * HDC], BF16, name=f"wqkv{i}") for i in range(ND)]
            wo_t = [wp.tile([P, D], BF16, name=f"wo{i}") for i in range(NM)]
            xts = {tn: [[wp.tile([P, 2 * CH], BF16, name=f"x{tn}_{hf}_{dd}")
                         for dd in range(ND)] for hf in range(2)]
                   for tn in ("v", "k", "q")}
            qT = [wp.tile([P, S], BF16, name=f"qT{m}") for m in range(NM)]
            kT = [wp.tile([P, S], BF16, name=f"kT{m}") for m in range(NM)]
            oU = [wp.tile([P, S], BF16, name=f"oU{m}") for m in range(NM)]
            oN = [wp.tile([P, S], BF16, name=f"oN{m}") for m in range(NM)]
            vaug = [wp.tile([P, HPC * VW], BF16, name=f"vaug{t}") for t in range(NST)]
            vTs = [wp.tile([P, CH], F32R, name=f"vTs{m}") for m in range(NM)]
            scr = wp.tile([1, 16], BF16, name="scr")

            # ---- DMA issue order (SP queue = strict FIFO priority) ----
            nc.sync.dma_start(idt[:], ident[:, :])
            nc.sync.dma_start(mA[:], maskA[:, :])
            nc.sync.dma_start(onb[:], onesb[:, :])
            nc.sync.dma_start(onp[:], onesp[:, :])
            xsrc = {"v": xvT, "k": xkT, "q": xqT}

            def dma_x(tn, hf, dd, eng):
                eng.dma_start(xts[tn][hf][dd][:],
                              xsrc[tn][dd * P:(dd + 1) * P,
                                       hf * 2 * CH:(hf + 1) * 2 * CH])

            for dd in range(ND):
                nc.sync.dma_start(wqkv_t[dd][:], wqkv[dd * P:(dd + 1) * P, :])
                dma_x("v", 0, dd, nc.sync)
            for dd in range(ND):
                dma_x("k", 0, dd, nc.sync)
            for dd in range(ND):
                dma_x("q", 0, dd, nc.sync)
            for i in range(NM):
                nc.sync.dma_start(wo_t[i][:], wo[i * P:(i + 1) * P, :])
            for tn in ("v", "k", "q"):
                for dd in range(ND):
                    dma_x(tn, 1, dd, nc.sync)

            # ACT: preload the Exp table during the DMA window
            nc.scalar.activation(scr[:], mA[0:1, 0:16], Exp)
            # vaug all-ones denominator columns (Pool memset; verified good)
            for t in range(NST):
                nc.gpsimd.memset(vaug[t][:, DV::VW], 1.0)

            # ---- emission helpers ----
            def dummy():
                dum = sc_ps.tile([P, P], F32R, name="dum", tag="sc")
                nc.tensor.transpose(dum[:], idt[:], idt[:])

            WSEL = {"q": 0, "k": HDC, "v": 2 * HDC}

            def proj_group(tn, m, c, dums=0):
                """project x_tn chunk c through W block m -> dest cols."""
                pp = aux_ps.tile([P, CH], F32, name="pg", tag="aux")
                hf, sub = c // 2, (c % 2) * CH
                for dd in range(ND):
                    w = wqkv_t[dd][:, WSEL[tn] + m * P:WSEL[tn] + (m + 1) * P]
                    nc.tensor.matmul(pp[:], w,
                                     xts[tn][hf][dd][:, sub:sub + CH],
                                     start=(dd == 0), stop=(dd == ND - 1))
                    for _ in range(dums):
                        dummy()
                if tn == "v":
                    nc.vector.tensor_copy(vTs[m][:], pp[:])
                else:
                    dst = (qT if tn == "q" else kT)[m][:, c * CH:(c + 1) * CH]
                    nc.vector.tensor_copy(dst, pp[:])

            def vtrans(m, c, j):
                """one [128,128] block of vTs[m] -> natural layout in vaug."""
                st = 4 * c + j
                tp = aux_ps.tile([P, P], F32R, name="tp", tag="aux")
                nc.tensor.transpose(tp[:], vTs[m][:, j * P:(j + 1) * P], idt[:])
                dst = vaug[st][:, m * 2 * VW:(m * 2 + 2) * VW].rearrange(
                    "p (h x) -> p h x", x=VW)[:, :, 0:DV]
                src = tp[:].rearrange("p (h x) -> p h x", x=DV)
                nc.vector.tensor_copy(dst, src)

            def sc_unit(h, c, t):
                """scores tile t for head h, chunk c -> exp'd ex tile."""
                mi, ri = h // 2, (h % 2) * DK
                r = t - 4 * c
                lo = max(r, 0) * P
                scp = sc_ps.tile([P, CH], F32, name="scp", tag="sc")
                nc.tensor.matmul(
                    scp[:, lo:CH],
                    kT[mi][ri:ri + DK, t * P:(t + 1) * P],
                    qT[mi][ri:ri + DK, c * CH + lo:(c + 1) * CH],
                    start=True, stop=True)
                ex = exp_.tile([P, CH], BF16, name="ex", tag="ex")
                nc.scalar.activation(ex[:, lo:CH], scp[:, lo:CH], Exp)
                if r >= 0:
                    eng = nc.vector if h < 2 else nc.gpsimd
                    eng.tensor_mul(ex[:, lo:lo + P], ex[:, lo:lo + P], mA[:])
                return ex, lo

            def ov_unit(ovt, h, c, t, ex, lo, nt):
                nc.tensor.matmul(ovt[:, lo:CH], vaug[t][:, h * VW:(h + 1) * VW],
                                 ex[:, lo:CH], start=(t == 0), stop=(t == nt - 1))

            recbs = {}

            def tail_recip(ovt, c, h, on_act):
                """recip of the denominator row -> recb (bf16)."""
                dn = sp.tile([1, CH], F32, name="dnf", tag="dnf", bufs=4)
                if on_act:
                    nc.scalar.copy(dn[:], ovt[DV:DV + 1, :])
                else:
                    nc.vector.tensor_copy(dn[:], ovt[DV:DV + 1, :])
                recf = sp.tile([1, CH], F32, name="recf", tag="recf", bufs=4)
                nc.vector.reciprocal_approx_fast(out=recf[:], in_=dn[:])
                rb = sp.tile([1, CH], BF16, name="recb", tag="recb", bufs=8)
                if on_act:
                    nc.scalar.copy(rb[:], recf[:])
                else:
                    nc.vector.tensor_copy(rb[:], recf[:])
                recbs[(c, h)] = rb
                if DEBUG:
                    r = 4 * c + h
                    nc.sync.dma_start(dbg["den"][r:r + 1, :], dn[:])
                    nc.sync.dma_start(dbg["rec"][r:r + 1, :], recf[:])

            def tail_oU(ovt, c, h, on_act):
                """stash the unnormalized numerator."""
                mi, ri = h // 2, (h % 2) * DK
                dst = oU[mi][ri:ri + DK, c * CH:(c + 1) * CH]
                if on_act:
                    nc.scalar.copy(dst, ovt[0:DV, :])
                else:
                    nc.vector.tensor_copy(dst, ovt[0:DV, :])

            def rank1mult(c, h):
                mi, ri = h // 2, (h % 2) * DK
                rb = aux_ps.tile([DK, CH], F32, name="rb", tag="aux")
                nc.tensor.matmul(rb[:], onb[0:1, 0:DK], recbs[(c, h)][:],
                                 start=True, stop=True)
                sl = slice(c * CH, (c + 1) * CH)
                nc.vector.tensor_mul(oN[mi][ri:ri + DK, sl],
                                     oU[mi][ri:ri + DK, sl], rb[:])

            def oproj(c, j):
                st = 4 * c + j
                ob = obp.tile([P, D], BF16, name="ob", tag="ob")
                for n2 in range(D // CH):
                    pp = sc_ps.tile([P, CH], F32, name="pout", tag="sc")
                    for m in range(NM):
                        nc.tensor.matmul(pp[:], oN[m][:, st * P:(st + 1) * P],
                                         wo_t[m][:, n2 * CH:(n2 + 1) * CH],
                                         start=(m == 0), stop=(m == NM - 1))
                    nc.vector.tensor_copy(ob[:, n2 * CH:(n2 + 1) * CH], pp[:])
                nc.sync.dma_start(out[st * P:(st + 1) * P, :], ob[:])

            # ---- W0: warmup + project chunk 0 ----
            for _ in range(PRE_DUMMIES):
                dummy()
            for i, tn in enumerate(("v", "k", "q")):
                proj_group(tn, 0, 0, dums=W0_DUMMIES[i])
                proj_group(tn, 1, 0, dums=2)
                if tn == "v":
                    for m in range(NM):
                        for j in range(4):
                            vtrans(m, 0, j)

            # ---- waves: chain(c) with interleaved fillers ----
            def wave_units(c):
                projs, rank1s, oprojs = [], [], []
                if c + 1 < NCH:
                    for tn in ("v", "k", "q"):
                        for m in range(NM):
                            projs.append(
                                lambda tn=tn, m=m: proj_group(tn, m, c + 1))
                        if tn == "v":
                            for m in range(NM):
                                for j in range(4):
                                    projs.append(
                                        lambda m=m, j=j: vtrans(m, c + 1, j))
                if c >= 1:
                    for h in range(HPC):
                        rank1s.append(lambda h=h: rank1mult(c - 1, h))
                    for j in range(4):
                        oprojs.append(lambda j=j: oproj(c - 1, j))
                return projs[:2] + rank1s + projs[2:] + oprojs

            for c in range(NCH):
                nt = 4 * c + 4
                last = c == NCH - 1
                units = wave_units(c)
                # hold back units to cover the chain tail's recip latency
                hold = 3 if last else 0
                done = 0
                ovts = [ov_ps.tile([DV + 1, CH], F32, name=f"ov{h}", tag="ov")
                        for h in range(HPC)]
                exq = []  # pending (t, [per-head (ex, lo)]) with lag 2
                for t in range(nt):
                    cur = [sc_unit(0, c, t), sc_unit(1, c, t)]
                    if len(exq) >= 2:
                        ot, oexs = exq.pop(0)
                        for h in range(HPC):
                            ov_unit(ovts[h], h, c, ot, *oexs[h], nt)
                    cur += [sc_unit(2, c, t), sc_unit(3, c, t)]
                    exq.append((t, cur))
                    want = min(len(units) * (t + 1) // nt, len(units) - hold)
                    while done < want:
                        units[done]()
                        done += 1
                for ot, oexs in exq:
                    for h in range(HPC):
                        ov_unit(ovts[h], h, c, ot, *oexs[h], nt)
                for h in range(HPC):
                    tail_recip(ovts[h], c, h, on_act=last)
                for h in range(HPC):
                    tail_oU(ovts[h], c, h, on_act=last)
                if last:
                    if done < len(units):
                        units[done]()
                        done += 1
                    for h in range(HPC):
                        rank1mult(c, h)
                while done < len(units):
                    units[done]()
                    done += 1

            # ---- drain ----
            for j in range(4):
                oproj(NCH - 1, j)
            if DEBUG:
                for n, t in (("qT0", qT[0]), ("kT0", kT[0]),
                             ("oU0", oU[0]), ("oN0", oN[0])):
                    nc.sync.dma_start(dbg[n][:, :], t[:])
    nc.compile()
    return nc


_NC_CACHE = {}
LAST_RESULT = None


def _get_nc():
    if "nc" not in _NC_CACHE:
        import concourse.tile as tile
        import concourse.mybir as mybir
        from concourse import bacc
        nc = bacc.Bacc("TRN2", target_bir_lowering=False, num_devices=NCORES)
        _NC_CACHE["nc"] = build(nc, tile, mybir)
    return _NC_CACHE["nc"]


def kernel(Q, K, V, Wq, Wk, Wv, Wo):
    import ml_dtypes
    from concourse.bass_utils import run_bass_kernel_spmd
    BF = ml_dtypes.bfloat16

    Q = np.asarray(Q, dtype=np.float32)
    K = np.asarray(K, dtype=np.float32)
    V = np.asarray(V, dtype=np.float32)
    Wq = np.asarray(Wq, dtype=np.float32) * np.float32(1.0 / np.sqrt(DK))
    Wk = np.asarray(Wk, dtype=np.float32)
    Wv = np.asarray(Wv, dtype=np.float32)
    Wo = np.asarray(Wo, dtype=np.float32)

    QT = [np.ascontiguousarray(Q[b].T).astype(BF) for b in range(B)]
    KT = [np.ascontiguousarray(K[b].T).astype(BF) for b in range(B)]
    VT = [np.ascontiguousarray(V[b].T).astype(BF) for b in range(B)]

    i = np.arange(P)[:, None]
    j = np.arange(P)[None, :]
    maskA = (j >= i).astype(BF)
    onesb = np.ones((1, P), dtype=BF)
    onesp = np.ones((P, HPC), dtype=BF)
    ident = np.eye(P, dtype=np.float32)

    in_maps = []
    for core in range(NCORES):
        b, g = core // HG, core % HG
        cs = slice(g * HDC, (g + 1) * HDC)
        in_maps.append({
            "xqT": QT[b], "xkT": KT[b], "xvT": VT[b],
            "wqkv": np.ascontiguousarray(
                np.concatenate([Wq[:, cs], Wk[:, cs], Wv[:, cs]],
                               axis=1)).astype(BF),
            "wo": np.ascontiguousarray(Wo[cs, :]).astype(BF),
            "maskA": maskA, "onesb": onesb, "onesp": onesp, "ident": ident,
        })

    nc = _get_nc()
    res = run_bass_kernel_spmd(nc, in_maps, core_ids=list(range(NCORES)))
    global LAST_RESULT
    LAST_RESULT = res

    acc = np.zeros((B, S, D), dtype=np.float64)
    for core in range(NCORES):
        acc[core // HG] += res.results[core]["out"].astype(np.float64)
    return acc.astype(np.float32)
